# revision 25
# baseline (speedup 1.0000x reference)
# Trainium2 Bass kernel: BIPA MultiHeadAttention (B=32, L=577, D=768, H=12)
# Data-parallel over batch: 4 batch items per NeuronCore x 8 cores.
#
# Per-core layout strategy (tokens padded 577 -> 640 = 5*128 per batch item;
# working width W = 580 = 2*290 even chunks for fp32r):
#   xT      [768, 2560]  host-transposed input (feature-major)
#   q,k     produced transposed:   qkT[feat, tok]   (lhsT = Wqk^T, rhs = xT)
#   v       produced normal:       v[tok, feat]     (lhsT = xT,    rhs = Wv^T)
#           with a ones-column interleaved per head ([v_h | 1] stride 65)
#   scores  computed transposed:   ST[j, i] = k_h^T q_h   per 128-key tile;
#           head pairs occupy PE row halves (rows 0:64 / 64:128) so their
#           score matmuls run concurrently on different row groups.
#           key bias (alpha*mb + pad mask) is per-PARTITION here, so
#           scale+bias+exp fuse into ONE ScalarE activation per tile.
#   AV      out^T[65, i] = [v_h | 1]^T @ exp(ST);  row 64 = softmax denominator
#   norm    pair-batched reciprocal, gpsimd partition-broadcast, DVE multiply
#   proj    lhsT = proj_w^T tile (stationary), rhs = rawT (moving)
#           -> projT[feat, tok] in PSUM, +bias per-partition on DVE
#   output  outT[768, 2560] -> host transposes/unpads/concats.
#
# Matmuls run as float32r (full PE rate, even moving dim >= 256); P/V run bf16.

import numpy as np

B, L, D = 32, 577, 768
H, HD = 12, 64
NCORES = 8
BPC = B // NCORES            # batch items per core
LP = 640                     # padded per-batch token count (5 * 128)
NJT = LP // 128              # key/token tiles per batch item
TOK = BPC * LP               # padded tokens per core
KT = D // 128                # contraction tiles over feature dim
NQK = 12                     # q+k output feature tiles (1536 / 128)
SCALE = HD ** -0.5
NEG = -30.0                  # pad-key bias (exp(-30) ~ 9e-14)
PSTRIDE = 512                # psum chunk stride (bank aligned)

_CACHE = {}


def _build(bpc=BPC, lp=LP, av_bf16=True, lr=None):
    import concourse.mybir as mybir
    import concourse.tile as tile
    from concourse import bacc

    f32 = mybir.dt.float32
    r32 = mybir.dt.float32r
    avdt = mybir.dt.bfloat16 if av_bf16 else f32
    njt = lp // 128
    tok = bpc * lp
    if lr is None:
        lr = L if lp == LP else lp
    ch = ((lr + 1) // 2 + 1) // 2 * 2       # even half-chunk (290 full size)
    W = 2 * ch                               # working token width (580)
    assert W <= lp
    CH = [(0, ch), (ch, W)]
    VCH = [(0, 384), (384, 768)]            # v feature chunks (6 heads each)
    NT_ORDER = [x for t in range(KT) for x in (t, KT + t)]  # q/k interleaved

    nc = bacc.Bacc(
        "TRN2",
        target_bir_lowering=False,
        debug=False,
        enable_asserts=False,
        num_devices=NCORES,
    )

    xT = nc.dram_tensor("xT", [D, tok], r32, kind="ExternalInput").ap()
    wqkvT = nc.dram_tensor("wqkvT", [D, 3 * D], r32, kind="ExternalInput").ap()
    pwT = nc.dram_tensor("pwT", [D, D], r32, kind="ExternalInput").ap()
    bqkT = nc.dram_tensor("bqkT", [128, NQK], f32, kind="ExternalInput").ap()
    bvr = nc.dram_tensor("bvr", [1, D], f32, kind="ExternalInput").ap()
    pbT = nc.dram_tensor("pbT", [128, KT], f32, kind="ExternalInput").ap()
    mbT = nc.dram_tensor("mbT", [128, bpc * njt], f32, kind="ExternalInput").ap()
    maskT = nc.dram_tensor("maskT", [128, bpc * njt], f32, kind="ExternalInput").ap()
    alphav = nc.dram_tensor("alphav", [1, 1], f32, kind="ExternalInput").ap()
    outT = nc.dram_tensor("outT", [D, tok], f32, kind="ExternalOutput").ap()

    def sv(ap):
        # strided 2-chunk view of a psum tile: [128, 2, ch] at stride PSTRIDE
        return ap.rearrange("p (c x) -> p c x", c=2)[:, :, 0:ch]

    def cv(ap):
        # contiguous 2-chunk view of a [.., W]-wide destination
        return ap.rearrange("p (c x) -> p c x", c=2)

    with tile.TileContext(nc) as tc:
        from contextlib import ExitStack

        with ExitStack() as ctx:
            wpool = ctx.enter_context(tc.tile_pool(name="w", bufs=1))
            cpool = ctx.enter_context(tc.tile_pool(name="c", bufs=1))
            xpool = ctx.enter_context(tc.tile_pool(name="x", bufs=1))
            qkpool = ctx.enter_context(tc.tile_pool(name="qk", bufs=2))
            vpool = ctx.enter_context(tc.tile_pool(name="v", bufs=1))
            ptpool = ctx.enter_context(tc.tile_pool(name="pt", bufs=4))
            rawpool = ctx.enter_context(tc.tile_pool(name="raw", bufs=1))
            rcpool = ctx.enter_context(tc.tile_pool(name="rc", bufs=1))
            opool = ctx.enter_context(tc.tile_pool(name="o", bufs=1))
            mmpool = ctx.enter_context(tc.tile_pool(name="mm", bufs=4, space="PSUM"))

            # ---- resident weights / constants ----
            wq = wpool.tile([128, KT, 3 * D], r32, tag="wq")
            nc.sync.dma_start(wq[:], wqkvT.rearrange("(t p) n -> p t n", p=128))
            pw = wpool.tile([128, KT, D], r32, tag="pw")
            nc.sync.dma_start(pw[:], pwT.rearrange("(t p) n -> p t n", p=128))

            bqk = cpool.tile([128, NQK], f32, tag="bqk")
            nc.sync.dma_start(bqk[:], bqkT)
            pb = cpool.tile([128, KT], f32, tag="pb")
            nc.sync.dma_start(pb[:], pbT)
            bvbc = cpool.tile([128, D], f32, tag="bvbc")
            nc.sync.dma_start(bvbc[:], bvr.to_broadcast([128, D]))
            albc = cpool.tile([128, 1], f32, tag="albc")
            nc.sync.dma_start(albc[:], alphav.to_broadcast([128, 1]))
            mbraw = cpool.tile([128, bpc * njt], f32, tag="mbraw")
            nc.sync.dma_start(mbraw[:], mbT)
            msk = cpool.tile([128, bpc * njt], f32, tag="msk")
            nc.sync.dma_start(msk[:], maskT)
            mbias = cpool.tile([128, bpc * njt], f32, tag="mbias")
            nc.vector.tensor_scalar_mul(mbias[:], mbraw[:], albc[:, 0:1])
            nc.vector.tensor_add(mbias[:], mbias[:], msk[:])

            xTr = xT.rearrange("(t p) m -> p t m", p=128)
            outTr = outT.rearrange("(t p) m -> p t m", p=128)

            for b in range(bpc):
                xb = xpool.tile([128, KT, lp], r32, tag="xb")
                nc.sync.dma_start(xb[:], xTr[:, :, b * lp:(b + 1) * lp])

                # ---- q/k projection (transposed layout) ----
                qk = qkpool.tile([128, NQK, W], r32, tag="qk")
                for nt in NT_ORDER:
                    ps = mmpool.tile([128, 1024], f32, tag="mm")
                    for kt in range(KT):
                        for ci, (c0, c1) in enumerate(CH):
                            nc.tensor.matmul(
                                ps[:, ci * PSTRIDE: ci * PSTRIDE + (c1 - c0)],
                                lhsT=wq[:, kt, nt * 128:(nt + 1) * 128],
                                rhs=xb[:, kt, c0:c1],
                                start=(kt == 0),
                                stop=(kt == KT - 1),
                            )
                    nc.vector.tensor_scalar_add(
                        cv(qk[:, nt, :]), sv(ps), bqk[:, nt:nt + 1])

                if lr < W:
                    # zero the k/q pad columns (read by the last-tile lhsT).
                    # memset can't target fp32r; DMA from xT's zero pad region.
                    zsrc = xTr[:, :, b * lp + lr: b * lp + W]
                    nc.sync.dma_start(qk[:, 0:KT, lr:W], zsrc)
                    nc.sync.dma_start(qk[:, KT:NQK, lr:W], zsrc)

                # ---- v projection (normal layout, 65-col stride per head) ----
                v = vpool.tile([128, njt, 12 * 65], avdt, tag="v")
                for mt in range(njt):
                    ps = mmpool.tile([128, 1024], f32, tag="mm")
                    for kt in range(KT):
                        for ci, (c0, c1) in enumerate(VCH):
                            nc.tensor.matmul(
                                ps[:, ci * PSTRIDE: ci * PSTRIDE + (c1 - c0)],
                                lhsT=xb[:, kt, mt * 128:(mt + 1) * 128],
                                rhs=wq[:, kt, 2 * D + c0: 2 * D + c1],
                                start=(kt == 0),
                                stop=(kt == KT - 1),
                            )
                    for ci, (c0, c1) in enumerate(VCH):
                        nc.vector.tensor_add(
                            v[:, mt, ci * 6 * 65:(ci + 1) * 6 * 65].rearrange(
                                "p (h e) -> p h e", h=6)[:, :, 0:64],
                            ps[:, ci * PSTRIDE: ci * PSTRIDE + 384].rearrange(
                                "p (h e) -> p h e", h=6),
                            bvbc[:, c0:c1].rearrange("p (h e) -> p h e", h=6),
                        )
                    nc.vector.memset(
                        v[:, mt, :].rearrange("p (h e) -> p h e", h=12)[:, :, 64:65],
                        1.0,
                    )

                # ---- attention, head pairs on PE row halves ----
                raw = rawpool.tile([128, KT, W], r32, tag="raw")
                for t in range(KT):
                    h0, h1 = 2 * t, 2 * t + 1
                    av0 = mmpool.tile([128, 1024], f32, tag="mm")
                    av1 = mmpool.tile([128, 1024], f32, tag="mm")
                    for jt in range(njt):
                        j0, j1 = jt * 128, min((jt + 1) * 128, W)
                        kk = j1 - j0
                        st0 = mmpool.tile([128, 1024], f32, tag="mm")
                        st1 = mmpool.tile([128, 1024], f32, tag="mm")
                        for ci, (c0, c1) in enumerate(CH):
                            nc.tensor.matmul(
                                st0[0:kk, ci * PSTRIDE: ci * PSTRIDE + (c1 - c0)],
                                lhsT=qk[0:64, KT + t, j0:j1],
                                rhs=qk[0:64, t, c0:c1],
                                start=True, stop=True)
                        for ci, (c0, c1) in enumerate(CH):
                            nc.tensor.matmul(
                                st1[0:kk, ci * PSTRIDE: ci * PSTRIDE + (c1 - c0)],
                                lhsT=qk[64:128, KT + t, j0:j1],
                                rhs=qk[64:128, t, c0:c1],
                                start=True, stop=True)
                        bias = mbias[:, b * njt + jt: b * njt + jt + 1]
                        pt0 = ptpool.tile([128, W], avdt, tag="pt")
                        nc.scalar.activation(
                            cv(pt0), sv(st0),
                            mybir.ActivationFunctionType.Exp,
                            bias=bias, scale=SCALE)
                        pt1 = ptpool.tile([128, W], avdt, tag="pt")
                        nc.scalar.activation(
                            cv(pt1), sv(st1),
                            mybir.ActivationFunctionType.Exp,
                            bias=bias, scale=SCALE)
                        for ci, (c0, c1) in enumerate(CH):
                            nc.tensor.matmul(
                                av0[0:65, ci * PSTRIDE: ci * PSTRIDE + (c1 - c0)],
                                lhsT=v[0:kk, jt, h0 * 65:(h0 + 1) * 65],
                                rhs=pt0[0:kk, c0:c1],
                                start=(jt == 0), stop=(jt == njt - 1),
                                skip_group_check=True)
                        for ci, (c0, c1) in enumerate(CH):
                            nc.tensor.matmul(
                                av1[0:65, ci * PSTRIDE: ci * PSTRIDE + (c1 - c0)],
                                lhsT=v[0:kk, jt, h1 * 65:(h1 + 1) * 65],
                                rhs=pt1[0:kk, c0:c1],
                                start=(jt == 0), stop=(jt == njt - 1),
                                skip_group_check=True)
                    # per-head softmax denominators (row 64 of av0/av1).
                    # custom-DVE ops misread PSUM: stage rows to SBUF first
                    # (TensorCopy may shift partition base; only 64->0 is
                    # validated on HW).
                    den0 = rcpool.tile([1, W], f32, tag="den0")
                    nc.vector.tensor_copy(cv(den0), sv(av0)[64:65])
                    den1 = rcpool.tile([1, W], f32, tag="den1")
                    nc.vector.tensor_copy(cv(den1), sv(av1)[64:65])
                    rc0 = rcpool.tile([1, W], f32, tag="rc0")
                    nc.vector.reciprocal_approx_fast(rc0[:], den0[:])
                    rc1 = rcpool.tile([1, W], f32, tag="rc1")
                    nc.vector.reciprocal_approx_fast(rc1[:], den1[:])
                    rcba = rcpool.tile([64, W], f32, tag="rcba")
                    nc.gpsimd.partition_broadcast(rcba[:], rc0[0:1, :])
                    rcbb = rcpool.tile([64, W], f32, tag="rcbb")
                    nc.gpsimd.partition_broadcast(rcbb[:], rc1[0:1, :])
                    nc.vector.tensor_mul(
                        cv(raw[0:64, t, :]), sv(av0)[0:64], cv(rcba[:]))
                    # DVE tensor_tensor requires aligned partitions: normalize
                    # h1 at rows 0:64 then DMA up to rows 64:128.
                    stg = rcpool.tile([64, W], r32, tag="stg")
                    nc.vector.tensor_mul(cv(stg[:]), sv(av1)[0:64], cv(rcbb[:]))
                    nc.sync.dma_start(raw[64:128, t, :], stg[:])

                # ---- output projection (transposed out) ----
                osb = opool.tile([128, KT, W], f32, tag="osb")
                for nt in range(KT):
                    ps = mmpool.tile([128, 1024], f32, tag="mm")
                    for kt in range(KT):
                        for ci, (c0, c1) in enumerate(CH):
                            nc.tensor.matmul(
                                ps[:, ci * PSTRIDE: ci * PSTRIDE + (c1 - c0)],
                                lhsT=pw[:, kt, nt * 128:(nt + 1) * 128],
                                rhs=raw[:, kt, c0:c1],
                                start=(kt == 0),
                                stop=(kt == KT - 1),
                            )
                    nc.vector.tensor_scalar_add(
                        cv(osb[:, nt, :]), sv(ps), pb[:, nt:nt + 1])
                nc.sync.dma_start(outTr[:, :, b * lp: b * lp + W], osb[:])

    nc.compile()
    return nc


def _host_prep(x, mb, qkv_w, qkv_b, proj_w, proj_b, alpha, bpc=BPC, lp=LP,
               ncores=NCORES, l=L):
    """Shard + lay out inputs. Returns in_maps (one dict per core)."""
    njt = lp // 128
    x = np.asarray(x, np.float32)
    mb = np.asarray(mb, np.float32)
    qkv_w = np.asarray(qkv_w, np.float32)
    qkv_b = np.asarray(qkv_b, np.float32)
    proj_w = np.asarray(proj_w, np.float32)
    proj_b = np.asarray(proj_b, np.float32)
    alpha = np.asarray(alpha, np.float32)

    wqkvT = np.ascontiguousarray(qkv_w.T)                      # [768, 2304]
    pwT = np.ascontiguousarray(proj_w.T)                       # [768, 768]
    bqkT = np.ascontiguousarray(qkv_b[:2 * D].reshape(NQK, 128).T)
    bvr = np.ascontiguousarray(qkv_b[2 * D:].reshape(1, D))
    pbT = np.ascontiguousarray(proj_b.reshape(KT, 128).T)
    alphav = alpha.reshape(1, 1)

    # pad-key mask, same for every batch item
    mask1 = np.zeros(lp, np.float32)
    mask1[l:] = NEG
    maskT = np.ascontiguousarray(
        np.tile(mask1.reshape(njt, 128), (bpc, 1)).reshape(bpc * njt, 128).T)

    in_maps = []
    for c in range(ncores):
        xb = x[c * bpc:(c + 1) * bpc]                          # [bpc, L, D]
        xp = np.zeros((bpc, lp, D), np.float32)
        xp[:, :l, :] = xb
        xTc = np.ascontiguousarray(xp.reshape(bpc * lp, D).T)  # [768, tok]

        mbb = mb[c * bpc:(c + 1) * bpc]                        # [bpc, L-1]
        mbp = np.zeros((bpc, lp), np.float32)
        mbp[:, 1:l] = mbb
        mbTc = np.ascontiguousarray(
            mbp.reshape(bpc * njt, 128).T)                     # [128, bpc*njt]

        in_maps.append({
            "xT": xTc, "wqkvT": wqkvT, "pwT": pwT, "bqkT": bqkT,
            "bvr": bvr, "pbT": pbT, "mbT": mbTc, "maskT": maskT,
            "alphav": alphav,
        })
    return in_maps


def _host_gather(outs, bpc=BPC, lp=LP, l=L):
    """outs: list of {'outT': [768, tok]} per core -> [B, L, D] fp32."""
    parts = []
    for o in outs:
        t = np.asarray(o["outT"]).T.reshape(bpc, lp, D)[:, :l, :]
        parts.append(t)
    return np.ascontiguousarray(np.concatenate(parts, 0)).astype(np.float32)


def kernel(x, mb, qkv_w, qkv_b, proj_w, proj_b, alpha):
    from concourse.bass_utils import run_bass_kernel_spmd

    if "nc" not in _CACHE:
        _CACHE["nc"] = _build()
    nc = _CACHE["nc"]
    in_maps = _host_prep(x, mb, qkv_w, qkv_b, proj_w, proj_b, alpha)
    res = run_bass_kernel_spmd(nc, in_maps, core_ids=list(range(NCORES)))
    return _host_gather(res.results)
